# revision 64
# baseline (speedup 1.0000x reference)
"""CrossAttention (DFFNet) Trainium2 Bass kernel.

Shapes (hardcoded): rgb/depth [4, 256, 64, 64] f32; Wq/Wk [32, 256]; Wv [256, 256].

    q = Wq @ d + bq          [B, 32, 4096]
    k = Wk @ d + bk          [B, 32, 4096]
    v = Wv @ r + bv          [B, 256, 4096]
    scores = q^T k           [B, 4096, 4096], softmax over keys (last dim)
    feat = v @ mask^T        [B, 256, 4096]

Sharding: 8 cores = 4 batches x 2 query-halves (2048 queries each). The host
rotates the key axis of d and r per core so the core's queries sit at columns
0:2048 (softmax + value reduction are permutation-invariant over keys), so a
single program serves all 8 cores.

Device layout: scores are computed TRANSPOSED, st[m, n] (keys m on partitions,
queries n free) so the feat matmul needs no transposes:
  - v^T[m, c] = r-slice^T @ Wv^T (bv is applied per-partition at the output
    stage, where channels sit on partitions; softmax rows sum to 1 so the
    value bias passes straight through feat).
  - feat[c, n] = sum_m v^T[m, c] * exp(st[m, n]) / S[n]
  - S[n]: DVE accumulates acc[:, n] += exp tiles (fp16, fast DVE mode);
    one tiny ones-matmul per query tile finishes the partition reduction.
    Keeps the big reduction OFF the tensor engine (~40us PE in the naive
    version).
  - no max-subtraction: |scores| < ~6, exp is well-conditioned.

Main loop runs in quad groups (4 key tiles per slot): the K=32 score matmuls
are 4-way row-packed (tile_position (32j, 0)) so all four stream concurrently
on disjoint PE row bands into two [128,1024] PSUM tiles; two [128,1024] exps
and 8 feat matmuls follow. Steady state is paced by the scalar engine's exp
(2 x 1.11us per quad) with PE at ~2.2us/quad — both ~95% busy. feat lags the
scores by two quads so the scores->exp->feat latency chain never stalls PE.
At query-tile boundaries fc is copied PSUM->SBUF immediately so the next
tile's accumulation starts without waiting for the softmax-normalize chain,
whose sums/broadcast matmuls are staggered across the following slots.

PSUM (8 banks): 3-deep [128,1024] rotation shared by scores/projection
units/sm/bc (6 banks) + two feat accumulators (2 banks).

Inputs are pre-cast to bf16 on the host and DMA'd in a few large transfers
(the single DMA queue costs ~0.65us per descriptor regardless of size, so
descriptor count dominates); q/k/v projections consume them as they land,
with bias adds alternating between the scalar and vector engines.
"""

import numpy as np
import ml_dtypes

import concourse.bacc as bacc
import concourse.bass as bass
import concourse.mybir as mybir
import concourse.tile as tile
from concourse.bass_utils import run_bass_kernel_spmd

B, C, H, W = 4, 256, 64, 64
HW = H * W            # 4096
CQK = 32
P = 128
NQ = HW // 2          # 2048 queries per core
NT = 512              # query tile
N_NT = NQ // NT       # 4
MT = HW // P          # 32 key tiles
KC = C // P           # 2 contraction tiles for the projections
NG = MT // 2          # 16 score groups (2 key-tiles each) per query tile
NTOT = N_NT * NG      # 64 groups

F32 = mybir.dt.float32
F16 = mybir.dt.float16
BF16 = mybir.dt.bfloat16
AF = mybir.ActivationFunctionType
OP = mybir.AluOpType
BF16_NP = ml_dtypes.bfloat16


def _emit(tc, io):
    nc = tc.nc
    d = io["d"].ap()          # [256, 4096] bf16 depth (keys + queries source)
    r = io["r"].ap()          # [256, 4096] bf16 rgb (values source)
    wcat = io["wcat"].ap()    # [128, 1024] bf16: both kc blocks of [Wq4T|Wk4T|WvT]
    bqk = io["bqk"].ap()      # [128, 2] f32 = [tile(bq,4) | tile(bk,4)]
    bvr = io["bvr"].ap()      # [128, 2] f32 = bv as per-c-half columns
    out = io["out"].ap()      # [256, 2048] f32

    from contextlib import ExitStack

    with ExitStack() as ctx:
        pw = ctx.enter_context(tc.tile_pool(name="weights", bufs=1))
        pin = ctx.enter_context(tc.tile_pool(name="inputs", bufs=1))
        pqk = ctx.enter_context(tc.tile_pool(name="qk", bufs=1))
        pvt = ctx.enter_context(tc.tile_pool(name="vt", bufs=1))
        pse = ctx.enter_context(tc.tile_pool(name="stexp", bufs=8))
        pacc = ctx.enter_context(tc.tile_pool(name="accp", bufs=2))
        pfcs = ctx.enter_context(tc.tile_pool(name="fcsb", bufs=4))
        psmall = ctx.enter_context(tc.tile_pool(name="small", bufs=2))
        pout = ctx.enter_context(tc.tile_pool(name="outsb", bufs=4))
        # PSUM: 8 banks of [128, 512] f32 total. Everything except the two
        # feat accumulators shares one 3-deep [128,1024] rotation (6 banks);
        # sm/bc/vp/bvp slot into the same rotation at their phase of the
        # schedule.
        ps_st = ctx.enter_context(
            tc.tile_pool(name="ps_st", bufs=3, space=bass.MemorySpace.PSUM))
        ps_feat = ctx.enter_context(     # fc0+fc1 [128,512] x2 = 2 banks
            tc.tile_pool(name="ps_feat", bufs=2, space=bass.MemorySpace.PSUM))
        ps_aux = ps_st

        # ---- weights: host packs both kc row-blocks side by side so ALL
        # weights arrive in a single DMA descriptor (~0.65us each on the
        # serialized queue).
        wc_sb = pw.tile([P, 1024], BF16, tag="wc")
        nc.sync.dma_start(wc_sb[:], wcat[:])
        wq_t = [wc_sb[:, kc * 512:kc * 512 + P] for kc in range(KC)]
        wk_t = [wc_sb[:, kc * 512 + P:kc * 512 + 2 * P] for kc in range(KC)]
        wv_t = [wc_sb[:, kc * 512 + 2 * P:kc * 512 + 2 * P + C]
                for kc in range(KC)]
        ones_row = pw.tile([1, P], BF16, tag="ones_row")
        nc.vector.memset(ones_row[:], 1.0)
        ones_col = pw.tile([P, 1], F16, tag="ones_col")
        nc.vector.memset(ones_col[:], 1.0)

        # ---- inputs: two half-row DMAs per [128, 4096] block, kc blocks
        # interleaved so each consumer unblocks earliest; bias DMAs slot in
        # after the first d halves.
        d_sb = [pin.tile([P, HW], BF16, tag=f"d{kc}", name=f"d{kc}")
                for kc in range(KC)]
        r_sb = [pin.tile([P, HW], BF16, tag=f"r{kc}", name=f"r{kc}")
                for kc in range(KC)]

        def _chunks(src, dst, ranges, first_then=None):
            for c0, c1 in ranges:
                for kc in range(KC):
                    nc.sync.dma_start(
                        dst[kc][:, c0:c1],
                        src[kc * P:(kc + 1) * P, c0:c1])
                if first_then is not None:
                    first_then()
                    first_then = None

        bqk_sb = pw.tile([P, 2], F32, tag="bqk")
        bq_sb = bqk_sb[:, 0:1]
        bk_sb = bqk_sb[:, 1:2]
        bvc = pw.tile([P, 2], F32, tag="bvc")
        _chunks(d, d_sb, ((0, 1024), (1024, 2048), (2048, 4096)),
                first_then=lambda: (
                    nc.sync.dma_start(bqk_sb[:], bqk[:]),
                    nc.sync.dma_start(bvc[:], bvr[:])))
        _chunks(r, r_sb, ((0, 2048), (2048, 4096)))

        # ---- q/k projections (4x-replicated): x4[32j+o, n] = x[o, n] ------
        # Fine-grained [128,512] PSUM units; the bias adds alternate between
        # the scalar and vector engines so neither serializes the chain.
        q4 = pqk.tile([P, NQ], BF16, tag="q4")
        k4 = pqk.tile([P, HW], BF16, tag="k4")
        units = [("q", q4, wq_t, bq_sb, s) for s in range(4)] + \
                [("k", k4, wk_t, bk_sb, s) for s in range(8)]

        def emit_unit(u, force_dve_bias=False):
            pref, dst, w_t, b_sb, s = units[u]
            g0 = s * NT
            # alternate PSUM pools (fc banks are idle during projections) so
            # the unit matmuls never wait on a bias-read 3 allocations back
            pool, ptag = (ps_st, "stp") if u % 2 == 0 else (ps_feat, "feat")
            pp = pool.tile([P, NT], F32, tag=ptag, name=f"{pref}p{s}")
            for kc in range(KC):
                nc.tensor.matmul(
                    pp[:],
                    lhsT=w_t[kc],
                    rhs=d_sb[kc][:, g0:g0 + NT],
                    start=(kc == 0),
                    stop=(kc == KC - 1),
                )
            if u % 2 == 0 and not force_dve_bias:
                nc.scalar.activation(
                    dst[:, g0:g0 + NT], pp[:], AF.Identity, bias=b_sb)
            else:
                nc.vector.tensor_scalar(
                    dst[:, g0:g0 + NT], pp[:], b_sb, None, OP.add)

        # Units for q (all) and k columns 0:1024 first -- enough for score
        # quads 0-1, whose scores+exps are hoisted here so the scalar
        # engine's exp chain starts ~4.5us earlier. The remaining k units'
        # bias adds are forced onto the vector engine so they never queue
        # AHEAD of the hoisted exps on the scalar engine.
        for u in range(6):
            emit_unit(u)

        # ---- v^T projection: vt[mt][p, c] = v[c, mt*128 + p] --------------
        # (bv is applied at the output stage where it is per-partition.)
        # Emitted INSIDE the first 8 main-loop slots (one 4-key-tile quad
        # each): the steady loop is paced by the scalar engine's exps, so the
        # vp matmuls ride in PE slack and the single [128,1024] PSUM->SBUF
        # copy per quad rides in DVE slack. One allocation per quad keeps the
        # shared PSUM rotation free of intra-slot dependencies.
        vtq_t = [None] * (MT // 4)

        def emit_vproj_quad(vq):
            vpq = ps_st.tile([P, 1024], F32, tag="stp", name=f"vpq{vq}")
            for j in range(4):
                mt = 4 * vq + j
                for kc in range(KC):
                    nc.tensor.matmul(
                        vpq[:, j * C:(j + 1) * C],
                        lhsT=r_sb[kc][:, mt * P:(mt + 1) * P],
                        rhs=wv_t[kc],
                        start=(kc == 0),
                        stop=(kc == KC - 1),
                    )
            t = pvt.tile([P, 1024], BF16, tag=f"vtq{vq}")
            nc.vector.tensor_copy(t[:], vpq[:])
            vtq_t[vq] = t

        def vt_slice(mt, h):
            return vtq_t[mt // 4][:, (mt % 4) * C + h * P:
                                  (mt % 4) * C + (h + 1) * P]

        # ---- main attention loop (quad groups: 4 key tiles per slot) ------
        NGQ = MT // 4            # 8 quads per query tile
        NTOTQ = N_NT * NGQ       # 32 quad slots
        acc_t = [None] * N_NT
        se_t = [None] * NTOTQ
        fc_t = [None] * N_NT     # PSUM accumulators (rotating 2 banks)
        fcs_t = [None] * N_NT    # SBUF copies
        sm_t = [None] * N_NT
        rc_t = [None] * N_NT
        bc_t = [None] * N_NT

        def emit_scores_exp(i):
            # Quad group: 4 key-tiles per slot, 4-way row-packed K=32
            # matmuls at tile_position (32j, 0) stream concurrently on
            # disjoint PE row bands, writing two [128,1024] PSUM tiles.
            nt, qq = divmod(i, NGQ)
            stp = [ps_st.tile([P, 1024], F32, tag="stp", name=f"stp{i}_{h}")
                   for h in range(2)]
            n0 = nt * NT
            for j in range(4):
                mt = 4 * qq + j
                nc.tensor.matmul(
                    stp[j // 2][:, (j % 2) * NT:(j % 2 + 1) * NT],
                    lhsT=k4[32 * j:32 * j + 32, mt * P:(mt + 1) * P],
                    rhs=q4[32 * j:32 * j + 32, n0:n0 + NT],
                    start=True,
                    stop=True,
                    tile_position=(32 * j, 0),
                )
            ses = []
            for h in range(2):
                se = pse.tile([P, 1024], BF16, tag="se", name=f"se{i}_{h}")
                nc.scalar.activation(se[:], stp[h][:], AF.Exp)
                ses.append(se)
            se_t[i] = ses

        def emit_acc(i):
            nt, qq = divmod(i, NGQ)
            if qq == 0:
                acc_t[nt] = pacc.tile([P, 1024], F16, tag="acc",
                                      name=f"acc{nt}")
                nc.vector.tensor_copy(acc_t[nt][:], se_t[i][0][:])
            else:
                nc.vector.tensor_tensor(acc_t[nt][:], acc_t[nt][:],
                                        se_t[i][0][:], OP.add)
            nc.vector.tensor_tensor(acc_t[nt][:], acc_t[nt][:],
                                    se_t[i][1][:], OP.add)

        def emit_feat(i):
            nt, qq = divmod(i, NGQ)
            if qq == 0:
                fc_t[nt] = [
                    ps_feat.tile([P, NT], F32, tag="feat",
                                 name=f"fc{nt}_{c}") for c in range(2)]
            fc = fc_t[nt]
            for j in range(4):
                mt = 4 * qq + j
                sej = se_t[i][j // 2][:, (j % 2) * NT:(j % 2 + 1) * NT]
                first = mt == 0
                last = mt == MT - 1
                nc.tensor.matmul(
                    fc[0][:], lhsT=vt_slice(mt, 0), rhs=sej,
                    start=first, stop=last,
                )
                nc.tensor.matmul(
                    fc[1][:], lhsT=vt_slice(mt, 1), rhs=sej,
                    start=first, stop=last,
                )
            se_t[i] = None

        def emit_fc_free(nt):
            # Copy fc out of PSUM right away so the next tile's accumulation
            # can claim the banks without waiting for the normalize chain.
            # One copy on the scalar engine, one on the vector engine: they
            # run in parallel, and the scalar engine idles at tile
            # boundaries anyway (its exp pipeline restarts with the tile).
            fcs_t[nt] = []
            for c in range(2):
                t = pfcs.tile([P, NT], F32, tag="fcs", name=f"fcs{nt}_{c}")
                if c == 0:
                    nc.scalar.copy(t[:], fc_t[nt][c][:])
                else:
                    nc.vector.tensor_copy(t[:], fc_t[nt][c][:])
                fcs_t[nt].append(t)

        def emit_fold(nt):
            accf = pacc.tile([P, NT], F16, tag="accf", name=f"accf{nt}")
            acc = acc_t[nt]
            nc.vector.tensor_tensor(
                accf[:], acc[:, 0:NT], acc[:, NT:1024], OP.add)
            acc_t[nt] = accf  # reuse slot to pass to emit_sm

        def emit_sm(nt):
            sm = ps_aux.tile([1, NT], F32, tag="stp", name=f"sm{nt}")
            nc.tensor.matmul(sm[:], lhsT=ones_col[:], rhs=acc_t[nt][:],
                             start=True, stop=True)
            sm_t[nt] = sm

        def emit_recip(nt):
            rc = psmall.tile([1, NT], F32, tag="recip")
            nc.vector.reciprocal_approx_fast(out=rc[:], in_=sm_t[nt][:])
            rc_h = psmall.tile([1, NT], BF16, tag="recip_h")
            nc.vector.tensor_copy(rc_h[:], rc[:])
            rc_t[nt] = rc_h

        def emit_bc(nt):
            bc = ps_aux.tile([P, NT], F32, tag="stp", name=f"bc{nt}")
            nc.tensor.matmul(bc[:], lhsT=ones_row[:], rhs=rc_t[nt][:],
                             start=True, stop=True)
            bc_t[nt] = bc

        def emit_norm_out(nt):
            n0 = nt * NT
            bc_sb = pout.tile([P, NT], F32, tag="bc_sb")
            nc.vector.tensor_copy(bc_sb[:], bc_t[nt][:])
            for c in range(2):
                tmp = pout.tile([P, NT], F32, tag="tmp")
                nc.vector.tensor_tensor(tmp[:], fcs_t[nt][c][:], bc_sb[:],
                                        OP.mult)
                ot = pout.tile([P, NT], F32, tag="ot")
                nc.vector.tensor_scalar(ot[:], tmp[:], bvc[:, c:c + 1], None,
                                        OP.add)
                nc.sync.dma_start(out[c * P:(c + 1) * P, n0:n0 + NT], ot[:])

        # Slot schedule: front work for quad i, feat lagging TWO quads so
        # the scores->exp->feat dependency round-trip spans three slots and
        # the loop is paced by pure engine work, not the latency chain. Tile
        # tails are staggered so no PE instruction waits on a long DVE chain.
        # Prepaid quads 0-1 + the rest of the projections under their exps.
        emit_scores_exp(0)
        emit_acc(0)
        emit_scores_exp(1)
        for u in range(6, len(units)):
            emit_unit(u, force_dve_bias=True)
        emit_vproj_quad(0)
        emit_acc(1)

        LAG = 2
        for i in range(2, NTOTQ + LAG + 2):
            if 1 <= i - 1 < MT // 4:
                emit_vproj_quad(i - 1)
            if i < NTOTQ:
                emit_scores_exp(i)
            # sm + recip run EARLY in their slot (PE: right after scores,
            # DVE: before everything else) so the sm PSUM tile's reader
            # finishes fast and never stalls the score-tile rotation.
            if i >= NGQ + LAG and (i - NGQ - LAG) % NGQ == 0:
                nt = (i - NGQ - LAG) // NGQ
                emit_sm(nt)
                emit_recip(nt)
            if LAG <= i <= NTOTQ + LAG - 1:
                if (i - LAG) % NGQ == NGQ - 1:
                    emit_fold((i - LAG) // NGQ)   # before the fc copies:
                emit_feat(i - LAG)                # sm-mm next slot needs it
                if (i - LAG) % NGQ == NGQ - 1:
                    emit_fc_free((i - LAG) // NGQ)
            if 2 <= i < NTOTQ:
                emit_acc(i)
            if i >= NGQ + LAG + 1 and (i - NGQ - LAG - 1) % NGQ == 0:
                nt = (i - NGQ - LAG - 1) // NGQ
                emit_bc(nt)
                emit_norm_out(nt)


_BUILT = None


def _build():
    global _BUILT
    if _BUILT is not None:
        return _BUILT
    nc = bacc.Bacc("TRN2", target_bir_lowering=False, debug=False)
    io = {
        "d": nc.dram_tensor("d", [C, HW], BF16, kind="ExternalInput"),
        "r": nc.dram_tensor("r", [C, HW], BF16, kind="ExternalInput"),
        "wcat": nc.dram_tensor("wcat", [P, 1024], BF16, kind="ExternalInput"),
        "bqk": nc.dram_tensor("bqk", [P, 2], F32, kind="ExternalInput"),
        "bvr": nc.dram_tensor("bvr", [P, 2], F32, kind="ExternalInput"),
        "out": nc.dram_tensor("out", [C, NQ], F32, kind="ExternalOutput"),
    }
    with tile.TileContext(nc) as tc:
        _emit(tc, io)
    nc.compile()
    _BUILT = nc
    return nc


def _in_maps(rgb, depth, Wq, bq, Wk, bk, Wv, bv):
    f = np.float32
    d_all = np.asarray(depth, f).reshape(B, C, HW).astype(BF16_NP)
    r_all = np.asarray(rgb, f).reshape(B, C, HW).astype(BF16_NP)
    wqt4 = np.tile(np.asarray(Wq, f).T, (1, 4))
    wkt4 = np.tile(np.asarray(Wk, f).T, (1, 4))
    wvt = np.asarray(Wv, f).T
    wcat_tall = np.concatenate([wqt4, wkt4, wvt], axis=1)  # [256, 512]
    # pack the two 128-row kc blocks side by side -> one [128, 1024] DMA
    wcat = np.ascontiguousarray(
        np.concatenate([wcat_tall[0:P], wcat_tall[P:C]],
                       axis=1).astype(BF16_NP))
    bqk = np.ascontiguousarray(
        np.stack([np.tile(np.asarray(bq, f), 4),
                  np.tile(np.asarray(bk, f), 4)], axis=1))
    bvr = np.ascontiguousarray(
        np.asarray(bv, f).reshape(2, P).T)  # [128, 2] per-c-half columns
    maps = []
    for core in range(8):
        b, half = core // 2, core % 2
        # Rotate the key axis so this core's query half sits at cols 0:2048;
        # softmax + the value reduction are permutation-invariant over keys
        # as long as d and r use the same rotation.
        rot = np.r_[half * NQ:(half * NQ + HW)] % HW
        maps.append({
            "d": np.ascontiguousarray(d_all[b][:, rot]),
            "r": np.ascontiguousarray(r_all[b][:, rot]),
            "wcat": wcat, "bqk": bqk, "bvr": bvr,
        })
    return maps


def kernel(rgb, depth, Wq, bq, Wk, bk, Wv, bv, **run_kwargs):
    nc = _build()
    maps = _in_maps(rgb, depth, Wq, bq, Wk, bk, Wv, bv)
    res = run_bass_kernel_spmd(nc, maps, core_ids=list(range(8)), **run_kwargs)
    results = res.results if hasattr(res, "results") else res
    out = np.empty((B, C, HW), dtype=np.float32)
    for core in range(8):
        b, half = core // 2, core % 2
        out[b][:, half * NQ:(half + 1) * NQ] = results[core]["out"]
    kernel.last_results = res
    return out.reshape(B, C, H, W)


# revision 65
# speedup vs baseline: 1.0056x; 1.0056x over previous
"""CrossAttention (DFFNet) Trainium2 Bass kernel.

Shapes (hardcoded): rgb/depth [4, 256, 64, 64] f32; Wq/Wk [32, 256]; Wv [256, 256].

    q = Wq @ d + bq          [B, 32, 4096]
    k = Wk @ d + bk          [B, 32, 4096]
    v = Wv @ r + bv          [B, 256, 4096]
    scores = q^T k           [B, 4096, 4096], softmax over keys (last dim)
    feat = v @ mask^T        [B, 256, 4096]

Sharding: 8 cores = 4 batches x 2 query-halves (2048 queries each). The host
rotates the key axis of d and r per core so the core's queries sit at columns
0:2048 (softmax + value reduction are permutation-invariant over keys), so a
single program serves all 8 cores.

Device layout: scores are computed TRANSPOSED, st[m, n] (keys m on partitions,
queries n free) so the feat matmul needs no transposes:
  - v^T[m, c] = r-slice^T @ Wv^T (bv is applied per-partition at the output
    stage, where channels sit on partitions; softmax rows sum to 1 so the
    value bias passes straight through feat).
  - feat[c, n] = sum_m v^T[m, c] * exp(st[m, n]) / S[n]
  - S[n]: DVE accumulates acc[:, n] += exp tiles (fp16, fast DVE mode);
    one tiny ones-matmul per query tile finishes the partition reduction.
    Keeps the big reduction OFF the tensor engine (~40us PE in the naive
    version).
  - no max-subtraction: |scores| < ~6, exp is well-conditioned.

Main loop runs in quad groups (4 key tiles per slot): the K=32 score matmuls
are 4-way row-packed (tile_position (32j, 0)) so all four stream concurrently
on disjoint PE row bands into two [128,1024] PSUM tiles; two [128,1024] exps
and 8 feat matmuls follow. Steady state is paced by the scalar engine's exp
(2 x 1.11us per quad) with PE at ~2.2us/quad — both ~95% busy. feat lags the
scores by two quads so the scores->exp->feat latency chain never stalls PE.
At query-tile boundaries fc is copied PSUM->SBUF immediately so the next
tile's accumulation starts without waiting for the softmax-normalize chain,
whose sums/broadcast matmuls are staggered across the following slots.

PSUM (8 banks): 3-deep [128,1024] rotation shared by scores/projection
units/sm/bc (6 banks) + two feat accumulators (2 banks).

Inputs are pre-cast to bf16 on the host and DMA'd in a few large transfers
(the single DMA queue costs ~0.65us per descriptor regardless of size, so
descriptor count dominates); q/k/v projections consume them as they land,
with bias adds alternating between the scalar and vector engines.
"""

import numpy as np
import ml_dtypes

import concourse.bacc as bacc
import concourse.bass as bass
import concourse.mybir as mybir
import concourse.tile as tile
from concourse.bass_utils import run_bass_kernel_spmd

B, C, H, W = 4, 256, 64, 64
HW = H * W            # 4096
CQK = 32
P = 128
NQ = HW // 2          # 2048 queries per core
NT = 512              # query tile
N_NT = NQ // NT       # 4
MT = HW // P          # 32 key tiles
KC = C // P           # 2 contraction tiles for the projections
NG = MT // 2          # 16 score groups (2 key-tiles each) per query tile
NTOT = N_NT * NG      # 64 groups

F32 = mybir.dt.float32
F16 = mybir.dt.float16
BF16 = mybir.dt.bfloat16
AF = mybir.ActivationFunctionType
OP = mybir.AluOpType
BF16_NP = ml_dtypes.bfloat16


def _emit(tc, io):
    nc = tc.nc
    d = io["d"].ap()          # [256, 4096] bf16 depth (keys + queries source)
    r = io["r"].ap()          # [256, 4096] bf16 rgb (values source)
    wcat = io["wcat"].ap()    # [128, 1024] bf16: both kc blocks of [Wq4T|Wk4T|WvT]
    bqk = io["bqk"].ap()      # [128, 2] f32 = [tile(bq,4) | tile(bk,4)]
    bvr = io["bvr"].ap()      # [128, 2] f32 = bv as per-c-half columns
    out = io["out"].ap()      # [256, 2048] f32

    from contextlib import ExitStack

    with ExitStack() as ctx:
        pw = ctx.enter_context(tc.tile_pool(name="weights", bufs=1))
        pin = ctx.enter_context(tc.tile_pool(name="inputs", bufs=1))
        pqk = ctx.enter_context(tc.tile_pool(name="qk", bufs=1))
        pvt = ctx.enter_context(tc.tile_pool(name="vt", bufs=1))
        pse = ctx.enter_context(tc.tile_pool(name="stexp", bufs=8))
        pacc = ctx.enter_context(tc.tile_pool(name="accp", bufs=2))
        pfcs = ctx.enter_context(tc.tile_pool(name="fcsb", bufs=4))
        psmall = ctx.enter_context(tc.tile_pool(name="small", bufs=2))
        pout = ctx.enter_context(tc.tile_pool(name="outsb", bufs=4))
        # PSUM: 8 banks of [128, 512] f32 total. Everything except the two
        # feat accumulators shares one 3-deep [128,1024] rotation (6 banks);
        # sm/bc/vp/bvp slot into the same rotation at their phase of the
        # schedule.
        ps_st = ctx.enter_context(
            tc.tile_pool(name="ps_st", bufs=3, space=bass.MemorySpace.PSUM))
        ps_feat = ctx.enter_context(     # fc0+fc1 [128,512] x2 = 2 banks
            tc.tile_pool(name="ps_feat", bufs=2, space=bass.MemorySpace.PSUM))
        ps_aux = ps_st

        # ---- weights: host packs both kc row-blocks side by side so ALL
        # weights arrive in a single DMA descriptor (~0.65us each on the
        # serialized queue).
        wc_sb = pw.tile([P, 1024], BF16, tag="wc")
        nc.sync.dma_start(wc_sb[:], wcat[:])
        wq_t = [wc_sb[:, kc * 512:kc * 512 + P] for kc in range(KC)]
        wk_t = [wc_sb[:, kc * 512 + P:kc * 512 + 2 * P] for kc in range(KC)]
        wv_t = [wc_sb[:, kc * 512 + 2 * P:kc * 512 + 2 * P + C]
                for kc in range(KC)]
        ones_row = pw.tile([1, P], BF16, tag="ones_row")
        nc.vector.memset(ones_row[:], 1.0)
        ones_col = pw.tile([P, 1], F16, tag="ones_col")
        nc.vector.memset(ones_col[:], 1.0)

        # ---- inputs: two half-row DMAs per [128, 4096] block, kc blocks
        # interleaved so each consumer unblocks earliest; bias DMAs slot in
        # after the first d halves.
        d_sb = [pin.tile([P, HW], BF16, tag=f"d{kc}", name=f"d{kc}")
                for kc in range(KC)]
        r_sb = [pin.tile([P, HW], BF16, tag=f"r{kc}", name=f"r{kc}")
                for kc in range(KC)]

        def _chunks(src, dst, ranges, first_then=None):
            for c0, c1 in ranges:
                for kc in range(KC):
                    nc.sync.dma_start(
                        dst[kc][:, c0:c1],
                        src[kc * P:(kc + 1) * P, c0:c1])
                if first_then is not None:
                    first_then()
                    first_then = None

        bqk_sb = pw.tile([P, 2], F32, tag="bqk")
        bq_sb = bqk_sb[:, 0:1]
        bk_sb = bqk_sb[:, 1:2]
        bvc = pw.tile([P, 2], F32, tag="bvc")
        _chunks(d, d_sb, ((0, 1024), (1024, 2048), (2048, 4096)),
                first_then=lambda: (
                    nc.sync.dma_start(bqk_sb[:], bqk[:]),
                    nc.sync.dma_start(bvc[:], bvr[:])))
        _chunks(r, r_sb, ((0, 2048), (2048, 4096)))

        # ---- q/k projections (4x-replicated): x4[32j+o, n] = x[o, n] ------
        # Fine-grained [128,512] PSUM units; the bias adds alternate between
        # the scalar and vector engines so neither serializes the chain.
        q4 = pqk.tile([P, NQ], BF16, tag="q4")
        k4 = pqk.tile([P, HW], BF16, tag="k4")
        units = [("q", q4, wq_t, bq_sb, s) for s in range(4)] + \
                [("k", k4, wk_t, bk_sb, s) for s in range(8)]
        for u, (pref, dst, w_t, b_sb, s) in enumerate(units):
            g0 = s * NT
            # alternate PSUM pools (fc banks are idle during projections) so
            # the unit matmuls never wait on a bias-read 3 allocations back
            pool, ptag = (ps_st, "stp") if u % 2 == 0 else (ps_feat, "feat")
            pp = pool.tile([P, NT], F32, tag=ptag, name=f"{pref}p{s}")
            for kc in range(KC):
                nc.tensor.matmul(
                    pp[:],
                    lhsT=w_t[kc],
                    rhs=d_sb[kc][:, g0:g0 + NT],
                    start=(kc == 0),
                    stop=(kc == KC - 1),
                )
            if u % 2 == 0:
                nc.scalar.activation(
                    dst[:, g0:g0 + NT], pp[:], AF.Identity, bias=b_sb)
            else:
                nc.vector.tensor_scalar(
                    dst[:, g0:g0 + NT], pp[:], b_sb, None, OP.add)

        # ---- v^T projection: vt[mt][p, c] = v[c, mt*128 + p] --------------
        # (bv is applied at the output stage where it is per-partition.)
        # Emitted INSIDE the first 8 main-loop slots (one 4-key-tile quad
        # each): the steady loop is paced by the scalar engine's exps, so the
        # vp matmuls ride in PE slack and the single [128,1024] PSUM->SBUF
        # copy per quad rides in DVE slack. One allocation per quad keeps the
        # shared PSUM rotation free of intra-slot dependencies.
        vtq_t = [None] * (MT // 4)

        def emit_vproj_quad(vq):
            vpq = ps_st.tile([P, 1024], F32, tag="stp", name=f"vpq{vq}")
            for j in range(4):
                mt = 4 * vq + j
                for kc in range(KC):
                    nc.tensor.matmul(
                        vpq[:, j * C:(j + 1) * C],
                        lhsT=r_sb[kc][:, mt * P:(mt + 1) * P],
                        rhs=wv_t[kc],
                        start=(kc == 0),
                        stop=(kc == KC - 1),
                    )
            t = pvt.tile([P, 1024], BF16, tag=f"vtq{vq}")
            nc.vector.tensor_copy(t[:], vpq[:])
            vtq_t[vq] = t

        def vt_slice(mt, h):
            return vtq_t[mt // 4][:, (mt % 4) * C + h * P:
                                  (mt % 4) * C + (h + 1) * P]

        # ---- main attention loop (quad groups: 4 key tiles per slot) ------
        NGQ = MT // 4            # 8 quads per query tile
        NTOTQ = N_NT * NGQ       # 32 quad slots
        acc_t = [None] * N_NT
        se_t = [None] * NTOTQ
        fc_t = [None] * N_NT     # PSUM accumulators (rotating 2 banks)
        fcs_t = [None] * N_NT    # SBUF copies
        sm_t = [None] * N_NT
        rc_t = [None] * N_NT
        bc_t = [None] * N_NT

        def emit_scores_exp(i):
            # Quad group: 4 key-tiles per slot, 4-way row-packed K=32
            # matmuls at tile_position (32j, 0) stream concurrently on
            # disjoint PE row bands, writing two [128,1024] PSUM tiles.
            nt, qq = divmod(i, NGQ)
            stp = [ps_st.tile([P, 1024], F32, tag="stp", name=f"stp{i}_{h}")
                   for h in range(2)]
            n0 = nt * NT
            for j in range(4):
                mt = 4 * qq + j
                nc.tensor.matmul(
                    stp[j // 2][:, (j % 2) * NT:(j % 2 + 1) * NT],
                    lhsT=k4[32 * j:32 * j + 32, mt * P:(mt + 1) * P],
                    rhs=q4[32 * j:32 * j + 32, n0:n0 + NT],
                    start=True,
                    stop=True,
                    tile_position=(32 * j, 0),
                )
            ses = []
            for h in range(2):
                se = pse.tile([P, 1024], BF16, tag="se", name=f"se{i}_{h}")
                nc.scalar.activation(se[:], stp[h][:], AF.Exp)
                ses.append(se)
            se_t[i] = ses

        def emit_acc(i):
            nt, qq = divmod(i, NGQ)
            if qq == 0:
                acc_t[nt] = pacc.tile([P, 1024], F16, tag="acc",
                                      name=f"acc{nt}")
                nc.vector.tensor_copy(acc_t[nt][:], se_t[i][0][:])
            else:
                nc.vector.tensor_tensor(acc_t[nt][:], acc_t[nt][:],
                                        se_t[i][0][:], OP.add)
            nc.vector.tensor_tensor(acc_t[nt][:], acc_t[nt][:],
                                    se_t[i][1][:], OP.add)

        def emit_feat(i):
            nt, qq = divmod(i, NGQ)
            if qq == 0:
                fc_t[nt] = [
                    ps_feat.tile([P, NT], F32, tag="feat",
                                 name=f"fc{nt}_{c}") for c in range(2)]
            fc = fc_t[nt]
            for j in range(4):
                mt = 4 * qq + j
                sej = se_t[i][j // 2][:, (j % 2) * NT:(j % 2 + 1) * NT]
                first = mt == 0
                last = mt == MT - 1
                nc.tensor.matmul(
                    fc[0][:], lhsT=vt_slice(mt, 0), rhs=sej,
                    start=first, stop=last,
                )
                nc.tensor.matmul(
                    fc[1][:], lhsT=vt_slice(mt, 1), rhs=sej,
                    start=first, stop=last,
                )
            se_t[i] = None

        def emit_fc_free(nt):
            # Copy fc out of PSUM right away so the next tile's accumulation
            # can claim the banks without waiting for the normalize chain.
            # One copy on the scalar engine, one on the vector engine: they
            # run in parallel, and the scalar engine idles at tile
            # boundaries anyway (its exp pipeline restarts with the tile).
            fcs_t[nt] = []
            for c in range(2):
                t = pfcs.tile([P, NT], F32, tag="fcs", name=f"fcs{nt}_{c}")
                if c == 0:
                    nc.scalar.copy(t[:], fc_t[nt][c][:])
                else:
                    nc.vector.tensor_copy(t[:], fc_t[nt][c][:])
                fcs_t[nt].append(t)

        def emit_fold(nt):
            accf = pacc.tile([P, NT], F16, tag="accf", name=f"accf{nt}")
            acc = acc_t[nt]
            nc.vector.tensor_tensor(
                accf[:], acc[:, 0:NT], acc[:, NT:1024], OP.add)
            acc_t[nt] = accf  # reuse slot to pass to emit_sm

        def emit_sm(nt):
            sm = ps_aux.tile([1, NT], F32, tag="stp", name=f"sm{nt}")
            nc.tensor.matmul(sm[:], lhsT=ones_col[:], rhs=acc_t[nt][:],
                             start=True, stop=True)
            sm_t[nt] = sm

        def emit_recip(nt):
            rc = psmall.tile([1, NT], F32, tag="recip")
            nc.vector.reciprocal_approx_fast(out=rc[:], in_=sm_t[nt][:])
            rc_h = psmall.tile([1, NT], BF16, tag="recip_h")
            nc.vector.tensor_copy(rc_h[:], rc[:])
            rc_t[nt] = rc_h

        def emit_bc(nt):
            bc = ps_aux.tile([P, NT], F32, tag="stp", name=f"bc{nt}")
            nc.tensor.matmul(bc[:], lhsT=ones_row[:], rhs=rc_t[nt][:],
                             start=True, stop=True)
            bc_t[nt] = bc

        def emit_norm_out(nt):
            n0 = nt * NT
            bc_sb = pout.tile([P, NT], F32, tag="bc_sb")
            nc.vector.tensor_copy(bc_sb[:], bc_t[nt][:])
            for c in range(2):
                tmp = pout.tile([P, NT], F32, tag="tmp")
                nc.vector.tensor_tensor(tmp[:], fcs_t[nt][c][:], bc_sb[:],
                                        OP.mult)
                ot = pout.tile([P, NT], F32, tag="ot")
                nc.vector.tensor_scalar(ot[:], tmp[:], bvc[:, c:c + 1], None,
                                        OP.add)
                nc.sync.dma_start(out[c * P:(c + 1) * P, n0:n0 + NT], ot[:])

        # Slot schedule: front work for quad i, feat lagging TWO quads so
        # the scores->exp->feat dependency round-trip spans three slots and
        # the loop is paced by pure engine work, not the latency chain. Tile
        # tails are staggered so no PE instruction waits on a long DVE chain.
        # Prepay quad 0: its scores+exp issue straight after the projection
        # units (the two stp tiles land at the tail of the unit rotation),
        # so the scalar engine's 71us exp chain starts ~5us earlier and the
        # pipeline warms up under the remaining projection work.
        emit_scores_exp(0)
        emit_acc(0)

        LAG = 2
        for i in range(1, NTOTQ + LAG + 2):
            if i - 1 < MT // 4:
                emit_vproj_quad(i - 1)
            if i < NTOTQ:
                emit_scores_exp(i)
            # sm + recip run EARLY in their slot (PE: right after scores,
            # DVE: before everything else) so the sm PSUM tile's reader
            # finishes fast and never stalls the score-tile rotation.
            if i >= NGQ + LAG and (i - NGQ - LAG) % NGQ == 0:
                nt = (i - NGQ - LAG) // NGQ
                emit_sm(nt)
                emit_recip(nt)
            if LAG <= i <= NTOTQ + LAG - 1:
                if (i - LAG) % NGQ == NGQ - 1:
                    emit_fold((i - LAG) // NGQ)   # before the fc copies:
                emit_feat(i - LAG)                # sm-mm next slot needs it
                if (i - LAG) % NGQ == NGQ - 1:
                    emit_fc_free((i - LAG) // NGQ)
            if 1 <= i < NTOTQ:
                emit_acc(i)
            if i >= NGQ + LAG + 1 and (i - NGQ - LAG - 1) % NGQ == 0:
                nt = (i - NGQ - LAG - 1) // NGQ
                emit_bc(nt)
                emit_norm_out(nt)


_BUILT = None


def _build():
    global _BUILT
    if _BUILT is not None:
        return _BUILT
    nc = bacc.Bacc("TRN2", target_bir_lowering=False, debug=False)
    io = {
        "d": nc.dram_tensor("d", [C, HW], BF16, kind="ExternalInput"),
        "r": nc.dram_tensor("r", [C, HW], BF16, kind="ExternalInput"),
        "wcat": nc.dram_tensor("wcat", [P, 1024], BF16, kind="ExternalInput"),
        "bqk": nc.dram_tensor("bqk", [P, 2], F32, kind="ExternalInput"),
        "bvr": nc.dram_tensor("bvr", [P, 2], F32, kind="ExternalInput"),
        "out": nc.dram_tensor("out", [C, NQ], F32, kind="ExternalOutput"),
    }
    with tile.TileContext(nc) as tc:
        _emit(tc, io)
    nc.compile()
    _BUILT = nc
    return nc


def _in_maps(rgb, depth, Wq, bq, Wk, bk, Wv, bv):
    f = np.float32
    d_all = np.asarray(depth, f).reshape(B, C, HW).astype(BF16_NP)
    r_all = np.asarray(rgb, f).reshape(B, C, HW).astype(BF16_NP)
    wqt4 = np.tile(np.asarray(Wq, f).T, (1, 4))
    wkt4 = np.tile(np.asarray(Wk, f).T, (1, 4))
    wvt = np.asarray(Wv, f).T
    wcat_tall = np.concatenate([wqt4, wkt4, wvt], axis=1)  # [256, 512]
    # pack the two 128-row kc blocks side by side -> one [128, 1024] DMA
    wcat = np.ascontiguousarray(
        np.concatenate([wcat_tall[0:P], wcat_tall[P:C]],
                       axis=1).astype(BF16_NP))
    bqk = np.ascontiguousarray(
        np.stack([np.tile(np.asarray(bq, f), 4),
                  np.tile(np.asarray(bk, f), 4)], axis=1))
    bvr = np.ascontiguousarray(
        np.asarray(bv, f).reshape(2, P).T)  # [128, 2] per-c-half columns
    maps = []
    for core in range(8):
        b, half = core // 2, core % 2
        # Rotate the key axis so this core's query half sits at cols 0:2048;
        # softmax + the value reduction are permutation-invariant over keys
        # as long as d and r use the same rotation.
        rot = np.r_[half * NQ:(half * NQ + HW)] % HW
        maps.append({
            "d": np.ascontiguousarray(d_all[b][:, rot]),
            "r": np.ascontiguousarray(r_all[b][:, rot]),
            "wcat": wcat, "bqk": bqk, "bvr": bvr,
        })
    return maps


def kernel(rgb, depth, Wq, bq, Wk, bk, Wv, bv, **run_kwargs):
    nc = _build()
    maps = _in_maps(rgb, depth, Wq, bq, Wk, bk, Wv, bv)
    res = run_bass_kernel_spmd(nc, maps, core_ids=list(range(8)), **run_kwargs)
    results = res.results if hasattr(res, "results") else res
    out = np.empty((B, C, HW), dtype=np.float32)
    for core in range(8):
        b, half = core // 2, core % 2
        out[b][:, half * NQ:(half + 1) * NQ] = results[core]["out"]
    kernel.last_results = res
    return out.reshape(B, C, H, W)


# revision 70
# speedup vs baseline: 1.0096x; 1.0039x over previous
"""CrossAttention (DFFNet) Trainium2 Bass kernel.

Shapes (hardcoded): rgb/depth [4, 256, 64, 64] f32; Wq/Wk [32, 256]; Wv [256, 256].

    q = Wq @ d + bq          [B, 32, 4096]
    k = Wk @ d + bk          [B, 32, 4096]
    v = Wv @ r + bv          [B, 256, 4096]
    scores = q^T k           [B, 4096, 4096], softmax over keys (last dim)
    feat = v @ mask^T        [B, 256, 4096]

Sharding: 8 cores = 4 batches x 2 query-halves (2048 queries each). The host
rotates the key axis of d and r per core so the core's queries sit at columns
0:2048 (softmax + value reduction are permutation-invariant over keys), so a
single program serves all 8 cores.

Device layout: scores are computed TRANSPOSED, st[m, n] (keys m on partitions,
queries n free) so the feat matmul needs no transposes:
  - v^T[m, c] = r-slice^T @ Wv^T (bv is applied per-partition at the output
    stage, where channels sit on partitions; softmax rows sum to 1 so the
    value bias passes straight through feat).
  - feat[c, n] = sum_m v^T[m, c] * exp(st[m, n]) / S[n]
  - S[n]: DVE accumulates acc[:, n] += exp tiles (fp16, fast DVE mode);
    one tiny ones-matmul per query tile finishes the partition reduction.
    Keeps the big reduction OFF the tensor engine (~40us PE in the naive
    version).
  - no max-subtraction: |scores| < ~6, exp is well-conditioned.

Main loop runs in quad groups (4 key tiles per slot): the K=32 score matmuls
are 4-way row-packed (tile_position (32j, 0)) so all four stream concurrently
on disjoint PE row bands into two [128,1024] PSUM tiles; two [128,1024] exps
and 8 feat matmuls follow. Steady state is paced by the scalar engine's exp
(2 x 1.11us per quad) with PE at ~2.2us/quad — both ~95% busy. feat lags the
scores by two quads so the scores->exp->feat latency chain never stalls PE.
At query-tile boundaries fc is copied PSUM->SBUF immediately so the next
tile's accumulation starts without waiting for the softmax-normalize chain,
whose sums/broadcast matmuls are staggered across the following slots.

PSUM (8 banks): 3-deep [128,1024] rotation shared by scores/projection
units/sm/bc (6 banks) + two feat accumulators (2 banks).

Inputs are pre-cast to bf16 on the host and DMA'd in a few large transfers
(the single DMA queue costs ~0.65us per descriptor regardless of size, so
descriptor count dominates); q/k/v projections consume them as they land,
with bias adds alternating between the scalar and vector engines.
"""

import numpy as np
import ml_dtypes

import concourse.bacc as bacc
import concourse.bass as bass
import concourse.mybir as mybir
import concourse.tile as tile
from concourse.bass_utils import run_bass_kernel_spmd

B, C, H, W = 4, 256, 64, 64
HW = H * W            # 4096
CQK = 32
P = 128
NQ = HW // 2          # 2048 queries per core
NT = 512              # query tile
N_NT = NQ // NT       # 4
MT = HW // P          # 32 key tiles
KC = C // P           # 2 contraction tiles for the projections
NG = MT // 2          # 16 score groups (2 key-tiles each) per query tile
NTOT = N_NT * NG      # 64 groups

F32 = mybir.dt.float32
F16 = mybir.dt.float16
BF16 = mybir.dt.bfloat16
AF = mybir.ActivationFunctionType
OP = mybir.AluOpType
BF16_NP = ml_dtypes.bfloat16


def _emit(tc, io):
    nc = tc.nc
    d = io["d"].ap()          # [256, 4096] bf16 depth (keys + queries source)
    r = io["r"].ap()          # [256, 4096] bf16 rgb (values source)
    wcat = io["wcat"].ap()    # [128, 1024] bf16: both kc blocks of [Wq4T|Wk4T|WvT]
    bqk = io["bqk"].ap()      # [128, 2] f32 = [tile(bq,4) | tile(bk,4)]
    bvr = io["bvr"].ap()      # [128, 2] f32 = bv as per-c-half columns
    out = io["out"].ap()      # [256, 2048] f32

    from contextlib import ExitStack

    with ExitStack() as ctx:
        pw = ctx.enter_context(tc.tile_pool(name="weights", bufs=1))
        pin = ctx.enter_context(tc.tile_pool(name="inputs", bufs=1))
        pqk = ctx.enter_context(tc.tile_pool(name="qk", bufs=1))
        pvt = ctx.enter_context(tc.tile_pool(name="vt", bufs=1))
        pse = ctx.enter_context(tc.tile_pool(name="stexp", bufs=4))
        pacc = ctx.enter_context(tc.tile_pool(name="accp", bufs=2))
        pfcs = ctx.enter_context(tc.tile_pool(name="fcsb", bufs=4))
        psmall = ctx.enter_context(tc.tile_pool(name="small", bufs=2))
        pout = ctx.enter_context(tc.tile_pool(name="outsb", bufs=4))
        # PSUM: 8 banks of [128, 512] f32 total. Everything except the two
        # feat accumulators shares one 3-deep [128,1024] rotation (6 banks);
        # sm/bc/vp/bvp slot into the same rotation at their phase of the
        # schedule.
        ps_st = ctx.enter_context(
            tc.tile_pool(name="ps_st", bufs=3, space=bass.MemorySpace.PSUM))
        ps_feat = ctx.enter_context(     # fc0+fc1 [128,512] x2 = 2 banks
            tc.tile_pool(name="ps_feat", bufs=2, space=bass.MemorySpace.PSUM))
        ps_aux = ps_st

        # ---- weights: host packs both kc row-blocks side by side so ALL
        # weights arrive in a single DMA descriptor (~0.65us each on the
        # serialized queue).
        wc_sb = pw.tile([P, 1024], BF16, tag="wc")
        nc.sync.dma_start(wc_sb[:], wcat[:])
        wq_t = [wc_sb[:, kc * 512:kc * 512 + P] for kc in range(KC)]
        wk_t = [wc_sb[:, kc * 512 + P:kc * 512 + 2 * P] for kc in range(KC)]
        wv_t = [wc_sb[:, kc * 512 + 2 * P:kc * 512 + 2 * P + C]
                for kc in range(KC)]
        ones_row = pw.tile([1, P], BF16, tag="ones_row")
        nc.vector.memset(ones_row[:], 1.0)
        ones_col = pw.tile([P, 1], F16, tag="ones_col")
        nc.vector.memset(ones_col[:], 1.0)

        # ---- inputs: two half-row DMAs per [128, 4096] block, kc blocks
        # interleaved so each consumer unblocks earliest; bias DMAs slot in
        # after the first d halves.
        d_sb = [pin.tile([P, HW], BF16, tag=f"d{kc}", name=f"d{kc}")
                for kc in range(KC)]
        r_sb = [pin.tile([P, HW], BF16, tag=f"r{kc}", name=f"r{kc}")
                for kc in range(KC)]

        def _chunks(src, dst, ranges, first_then=None):
            for c0, c1 in ranges:
                for kc in range(KC):
                    nc.sync.dma_start(
                        dst[kc][:, c0:c1],
                        src[kc * P:(kc + 1) * P, c0:c1])
                if first_then is not None:
                    first_then()
                    first_then = None

        bqk_sb = pw.tile([P, 2], F32, tag="bqk")
        bq_sb = bqk_sb[:, 0:1]
        bk_sb = bqk_sb[:, 1:2]
        bvc = pw.tile([P, 2], F32, tag="bvc")
        _chunks(d, d_sb, ((0, 1024), (1024, 2048), (2048, 4096)),
                first_then=lambda: (
                    nc.sync.dma_start(bqk_sb[:], bqk[:]),
                    nc.sync.dma_start(bvc[:], bvr[:])))
        _chunks(r, r_sb, ((0, 2048), (2048, 4096)))

        # ---- q/k projections (4x-replicated): x4[32j+o, n] = x[o, n] ------
        # Fine-grained [128,512] PSUM units; the bias adds alternate between
        # the scalar and vector engines so neither serializes the chain.
        q4 = pqk.tile([P, NQ], BF16, tag="q4")
        k4 = pqk.tile([P, HW], BF16, tag="k4")
        units = [("q", q4, wq_t, bq_sb, s) for s in range(4)] + \
                [("k", k4, wk_t, bk_sb, s) for s in range(8)]
        for u, (pref, dst, w_t, b_sb, s) in enumerate(units):
            g0 = s * NT
            # alternate PSUM pools (fc banks are idle during projections) so
            # the unit matmuls never wait on a bias-read 3 allocations back
            pool, ptag = (ps_st, "stp") if u % 2 == 0 else (ps_feat, "feat")
            pp = pool.tile([P, NT], F32, tag=ptag, name=f"{pref}p{s}")
            for kc in range(KC):
                nc.tensor.matmul(
                    pp[:],
                    lhsT=w_t[kc],
                    rhs=d_sb[kc][:, g0:g0 + NT],
                    start=(kc == 0),
                    stop=(kc == KC - 1),
                )
            if u % 2 == 0:
                nc.scalar.activation(
                    dst[:, g0:g0 + NT], pp[:], AF.Identity, bias=b_sb)
            else:
                nc.vector.tensor_scalar(
                    dst[:, g0:g0 + NT], pp[:], b_sb, None, OP.add)

        # ---- v^T projection: vt[mt][p, c] = v[c, mt*128 + p] --------------
        # (bv is applied at the output stage where it is per-partition.)
        # Emitted INSIDE the first 8 main-loop slots (one 4-key-tile quad
        # each): the steady loop is paced by the scalar engine's exps, so the
        # vp matmuls ride in PE slack and the single [128,1024] PSUM->SBUF
        # copy per quad rides in DVE slack. One allocation per quad keeps the
        # shared PSUM rotation free of intra-slot dependencies.
        vtq_t = [None] * (MT // 4)

        def emit_vproj_quad(vq):
            vpq = ps_st.tile([P, 1024], F32, tag="stp", name=f"vpq{vq}")
            for j in range(4):
                mt = 4 * vq + j
                for kc in range(KC):
                    nc.tensor.matmul(
                        vpq[:, j * C:(j + 1) * C],
                        lhsT=r_sb[kc][:, mt * P:(mt + 1) * P],
                        rhs=wv_t[kc],
                        start=(kc == 0),
                        stop=(kc == KC - 1),
                    )
            t = pvt.tile([P, 1024], BF16, tag=f"vtq{vq}")
            nc.vector.tensor_copy(t[:], vpq[:])
            vtq_t[vq] = t

        def vt_slice(mt, h):
            return vtq_t[mt // 4][:, (mt % 4) * C + h * P:
                                  (mt % 4) * C + (h + 1) * P]

        # ---- main attention loop (quad groups: 4 key tiles per slot) ------
        NGQ = MT // 4            # 8 quads per query tile
        NTOTQ = N_NT * NGQ       # 32 quad slots
        acc_t = [None] * N_NT
        se_t = [None] * NTOTQ
        fc_t = [None] * N_NT     # PSUM accumulators (rotating 2 banks)
        fcs_t = [None] * N_NT    # SBUF copies
        sm_t = [None] * N_NT
        rc_t = [None] * N_NT
        bc_t = [None] * N_NT

        def emit_scores_exp(i):
            # Quad group: 4 key-tiles per slot, 4-way row-packed K=32
            # matmuls at tile_position (32j, 0) stream concurrently on
            # disjoint PE row bands, writing two [128,1024] PSUM tiles.
            nt, qq = divmod(i, NGQ)
            stp = [ps_st.tile([P, 1024], F32, tag="stp", name=f"stp{i}_{h}")
                   for h in range(2)]
            n0 = nt * NT
            for j in range(4):
                mt = 4 * qq + j
                nc.tensor.matmul(
                    stp[j // 2][:, (j % 2) * NT:(j % 2 + 1) * NT],
                    lhsT=k4[32 * j:32 * j + 32, mt * P:(mt + 1) * P],
                    rhs=q4[32 * j:32 * j + 32, n0:n0 + NT],
                    start=True,
                    stop=True,
                    tile_position=(32 * j, 0),
                )
            se = pse.tile([P, 2048], BF16, tag="se", name=f"se{i}")
            for h in range(2):
                nc.scalar.activation(se[:, h * 1024:(h + 1) * 1024],
                                     stp[h][:], AF.Exp)
            se_t[i] = se

        def emit_acc(i):
            # acc is [128, 2048] fp16 viewed as two 1024 halves; one DVE op
            # per quad covers the whole merged se tile.
            nt, qq = divmod(i, NGQ)
            if qq == 0:
                acc_t[nt] = pacc.tile([P, 2048], F16, tag="acc",
                                      name=f"acc{nt}")
                nc.vector.tensor_copy(acc_t[nt][:], se_t[i][:])
            else:
                nc.vector.tensor_tensor(acc_t[nt][:], acc_t[nt][:],
                                        se_t[i][:], OP.add)

        def emit_feat(i):
            nt, qq = divmod(i, NGQ)
            if qq == 0:
                fc_t[nt] = [
                    ps_feat.tile([P, NT], F32, tag="feat",
                                 name=f"fc{nt}_{c}") for c in range(2)]
            fc = fc_t[nt]
            for j in range(4):
                mt = 4 * qq + j
                sej = se_t[i][:, (j // 2) * 1024 + (j % 2) * NT:
                              (j // 2) * 1024 + (j % 2 + 1) * NT]
                first = mt == 0
                last = mt == MT - 1
                nc.tensor.matmul(
                    fc[0][:], lhsT=vt_slice(mt, 0), rhs=sej,
                    start=first, stop=last,
                )
                nc.tensor.matmul(
                    fc[1][:], lhsT=vt_slice(mt, 1), rhs=sej,
                    start=first, stop=last,
                )
            se_t[i] = None

        def emit_fc_free(nt):
            # Copy fc out of PSUM right away so the next tile's accumulation
            # can claim the banks without waiting for the normalize chain.
            # One copy on the scalar engine, one on the vector engine: they
            # run in parallel, and the scalar engine idles at tile
            # boundaries anyway (its exp pipeline restarts with the tile).
            fcs_t[nt] = []
            for c in range(2):
                t = pfcs.tile([P, NT], F32, tag="fcs", name=f"fcs{nt}_{c}")
                if c == 0:
                    nc.scalar.copy(t[:], fc_t[nt][c][:])
                else:
                    nc.vector.tensor_copy(t[:], fc_t[nt][c][:])
                fcs_t[nt].append(t)

        def emit_fold(nt):
            accf = pacc.tile([P, 1024], F16, tag="accf", name=f"accf{nt}")
            acc = acc_t[nt]
            nc.vector.tensor_tensor(
                accf[:], acc[:, 0:1024], acc[:, 1024:2048], OP.add)
            acc_t[nt] = accf  # reuse slot to pass to emit_sm

        def emit_sm(nt):
            # two accumulating 512-col matmuls finish the 4-way column fold
            sm = ps_aux.tile([1, NT], F32, tag="stp", name=f"sm{nt}")
            accf = acc_t[nt]
            nc.tensor.matmul(sm[:], lhsT=ones_col[:], rhs=accf[:, 0:NT],
                             start=True, stop=False)
            nc.tensor.matmul(sm[:], lhsT=ones_col[:], rhs=accf[:, NT:1024],
                             start=False, stop=True)
            sm_t[nt] = sm

        def emit_recip(nt):
            rc = psmall.tile([1, NT], F32, tag="recip")
            nc.vector.reciprocal_approx_fast(out=rc[:], in_=sm_t[nt][:])
            rc_h = psmall.tile([1, NT], BF16, tag="recip_h")
            nc.vector.tensor_copy(rc_h[:], rc[:])
            rc_t[nt] = rc_h

        def emit_bc(nt):
            bc = ps_aux.tile([P, NT], F32, tag="stp", name=f"bc{nt}")
            nc.tensor.matmul(bc[:], lhsT=ones_row[:], rhs=rc_t[nt][:],
                             start=True, stop=True)
            bc_t[nt] = bc

        def emit_norm_out(nt):
            n0 = nt * NT
            bc_sb = pout.tile([P, NT], F32, tag="bc_sb")
            nc.vector.tensor_copy(bc_sb[:], bc_t[nt][:])
            for c in range(2):
                tmp = pout.tile([P, NT], F32, tag="tmp")
                nc.vector.tensor_tensor(tmp[:], fcs_t[nt][c][:], bc_sb[:],
                                        OP.mult)
                ot = pout.tile([P, NT], F32, tag="ot")
                nc.vector.tensor_scalar(ot[:], tmp[:], bvc[:, c:c + 1], None,
                                        OP.add)
                nc.sync.dma_start(out[c * P:(c + 1) * P, n0:n0 + NT], ot[:])

        # Slot schedule: front work for quad i, feat lagging TWO quads so
        # the scores->exp->feat dependency round-trip spans three slots and
        # the loop is paced by pure engine work, not the latency chain. Tile
        # tails are staggered so no PE instruction waits on a long DVE chain.
        # Prepay quad 0: its scores+exp issue straight after the projection
        # units (the two stp tiles land at the tail of the unit rotation),
        # so the scalar engine's 71us exp chain starts ~5us earlier and the
        # pipeline warms up under the remaining projection work.
        emit_scores_exp(0)
        emit_acc(0)

        LAG = 2
        for i in range(1, NTOTQ + LAG + 2):
            if i - 1 < MT // 4:
                emit_vproj_quad(i - 1)
            if i < NTOTQ:
                emit_scores_exp(i)
            # sm + recip run EARLY in their slot (PE: right after scores,
            # DVE: before everything else) so the sm PSUM tile's reader
            # finishes fast and never stalls the score-tile rotation.
            if i >= NGQ + LAG and (i - NGQ - LAG) % NGQ == 0:
                nt = (i - NGQ - LAG) // NGQ
                emit_sm(nt)
                emit_recip(nt)
            if LAG <= i <= NTOTQ + LAG - 1:
                if (i - LAG) % NGQ == NGQ - 1:
                    emit_fold((i - LAG) // NGQ)   # before the fc copies:
                emit_feat(i - LAG)                # sm-mm next slot needs it
                if (i - LAG) % NGQ == NGQ - 1:
                    emit_fc_free((i - LAG) // NGQ)
            if 1 <= i < NTOTQ:
                emit_acc(i)
            if i >= NGQ + LAG + 1 and (i - NGQ - LAG - 1) % NGQ == 0:
                nt = (i - NGQ - LAG - 1) // NGQ
                emit_bc(nt)
                emit_norm_out(nt)


_BUILT = None


def _build():
    global _BUILT
    if _BUILT is not None:
        return _BUILT
    nc = bacc.Bacc("TRN2", target_bir_lowering=False, debug=False)
    io = {
        "d": nc.dram_tensor("d", [C, HW], BF16, kind="ExternalInput"),
        "r": nc.dram_tensor("r", [C, HW], BF16, kind="ExternalInput"),
        "wcat": nc.dram_tensor("wcat", [P, 1024], BF16, kind="ExternalInput"),
        "bqk": nc.dram_tensor("bqk", [P, 2], F32, kind="ExternalInput"),
        "bvr": nc.dram_tensor("bvr", [P, 2], F32, kind="ExternalInput"),
        "out": nc.dram_tensor("out", [C, NQ], F32, kind="ExternalOutput"),
    }
    with tile.TileContext(nc) as tc:
        _emit(tc, io)
    nc.compile()
    _BUILT = nc
    return nc


def _in_maps(rgb, depth, Wq, bq, Wk, bk, Wv, bv):
    f = np.float32
    d_all = np.asarray(depth, f).reshape(B, C, HW).astype(BF16_NP)
    r_all = np.asarray(rgb, f).reshape(B, C, HW).astype(BF16_NP)
    wqt4 = np.tile(np.asarray(Wq, f).T, (1, 4))
    wkt4 = np.tile(np.asarray(Wk, f).T, (1, 4))
    wvt = np.asarray(Wv, f).T
    wcat_tall = np.concatenate([wqt4, wkt4, wvt], axis=1)  # [256, 512]
    # pack the two 128-row kc blocks side by side -> one [128, 1024] DMA
    wcat = np.ascontiguousarray(
        np.concatenate([wcat_tall[0:P], wcat_tall[P:C]],
                       axis=1).astype(BF16_NP))
    bqk = np.ascontiguousarray(
        np.stack([np.tile(np.asarray(bq, f), 4),
                  np.tile(np.asarray(bk, f), 4)], axis=1))
    bvr = np.ascontiguousarray(
        np.asarray(bv, f).reshape(2, P).T)  # [128, 2] per-c-half columns
    maps = []
    for core in range(8):
        b, half = core // 2, core % 2
        # Rotate the key axis so this core's query half sits at cols 0:2048;
        # softmax + the value reduction are permutation-invariant over keys
        # as long as d and r use the same rotation.
        rot = np.r_[half * NQ:(half * NQ + HW)] % HW
        maps.append({
            "d": np.ascontiguousarray(d_all[b][:, rot]),
            "r": np.ascontiguousarray(r_all[b][:, rot]),
            "wcat": wcat, "bqk": bqk, "bvr": bvr,
        })
    return maps


def kernel(rgb, depth, Wq, bq, Wk, bk, Wv, bv, **run_kwargs):
    nc = _build()
    maps = _in_maps(rgb, depth, Wq, bq, Wk, bk, Wv, bv)
    res = run_bass_kernel_spmd(nc, maps, core_ids=list(range(8)), **run_kwargs)
    results = res.results if hasattr(res, "results") else res
    out = np.empty((B, C, HW), dtype=np.float32)
    for core in range(8):
        b, half = core // 2, core % 2
        out[b][:, half * NQ:(half + 1) * NQ] = results[core]["out"]
    kernel.last_results = res
    return out.reshape(B, C, H, W)
